# revision 22
# baseline (speedup 1.0000x reference)
"""Multi-head self-attention (no mask) for Trainium2, distributed over 8 NeuronCores.

Problem (hardcoded): src [4, 2048, 512] f32, Wq/Wk/Wv [512, 512], bq/bk/bv [512],
H=8 heads of dim 64.  out = softmax(Q K^T / 8) V reshaped to [4, 2048, 512].

Sharding: 8 cores = 4 batches x 2 head-groups (4 heads each); each core computes
its own QKV projection for its 256 feature columns from the host-pre-transposed,
host-pre-bf16 src[b]^T.

The kernel is paced by exp over all scores (16.8M elements/core).  Structure:
  - unit = (head pair, 512-wide q block); per key chunk kc one [128,1024] PSUM
    score tile holds both heads' scores (row-split matmul pair), one exp, two
    attnV matmuls accumulating into [65,512] PSUM tiles (V carries a ones
    column so row 64 accumulates the softmax denominator).
  - flat software pipeline over 8 units x 16 kc: attnV trails scores/exp by one
    step so unit boundaries never stall the ACT engine.
  - exp runs on ACT, except every 8th step computed on DVE via the Schraudolph
    bit-trick (x*A+B rounded to int16 IS the bf16 pattern of e^x), splitting
    the exp workload across two engines.
  - QKV projection + per-unit finalize drip into the attention loop's PE slack
    through two spare PSUM banks (8 = 2x2 score + 2 acc + 2 jit).
  - finalize per (unit, head): copy acc to SBUF, reciprocal of denominator row,
    broadcast it over 64 partitions with a rank-1 f32r matmul, one
    tensor_tensor multiply, DMA the [d, q] tile to a d-major output which the
    host transposes.
"""

import numpy as np

import concourse.bass as bass
import concourse.tile as tile
from concourse import bacc, masks, mybir
from concourse.bass_utils import run_bass_kernel_spmd

B, S, D = 4, 2048, 512
H = 8
HD = 64
N_CORES = 8
HPC = 4            # heads per core
CW = HPC * HD      # feature columns per core (256)
NKC = S // 128     # key chunks (16)
NQT = S // 512     # query tiles (4)
SCALE = 1.0 / 8.0  # 1/sqrt(HD)

F32 = mybir.dt.float32
F32R = mybir.dt.float32r
BF16 = mybir.dt.bfloat16
I16 = mybir.dt.int16
BF16_NP = mybir.dt.np(mybir.dt.bfloat16)

# Schraudolph exp-as-bf16-bits: rint(x*SCALE*2^7/ln2 + B) viewed as bf16 is
# e^(x*SCALE) to within 3.3% max / 2.0% rms (B tuned for round-to-nearest,
# which is what the DVE float->int16 convert does).
EXP_A = 184.6650390625 * SCALE
EXP_B = 16250.375
# steps whose exp runs on DVE instead of ACT
OFF_STEPS = frozenset(range(20, 128, 8))


def _body(tc, srcT, wq, wk, wv, bq, bk, bv, out_d):
    nc = tc.nc
    with (
        tc.tile_pool(name="const", bufs=1) as const,
        tc.tile_pool(name="persist", bufs=1) as persist,
        tc.tile_pool(name="expp", bufs=6) as expp,
        tc.tile_pool(name="fin", bufs=3) as fin,
        tc.tile_pool(name="psumS", bufs=1, space="PSUM") as psumS,
        tc.tile_pool(name="psumA", bufs=1, space="PSUM") as psumA,
    ):
        ident = const.tile([128, 128], F32, name="ident")
        masks.make_identity(nc, ident)
        ones_row = const.tile([1, 128], BF16)
        nc.vector.memset(ones_row, 1.0)

        # --- input DMA, ordered so the first score matmul unblocks ASAP ---
        W = {}
        wb = persist.tile([128, 4, CW], BF16, tag="Wwq", name="wq")
        nc.sync.dma_start(out=wb, in_=wq.rearrange("c p n -> p c n"))
        W["wq"] = wb
        src_v = srcT.rearrange("(c p) s -> p c s", p=128)
        srcT_bf = persist.tile([128, 4, S], BF16, tag="srcT", name="srcT")
        nc.sync.dma_start(out=srcT_bf[:, :, 0:512], in_=src_v[:, :, 0:512])
        wb = persist.tile([128, 4, CW], BF16, tag="Wwk", name="wk")
        nc.sync.dma_start(out=wb, in_=wk.rearrange("c p n -> p c n"))
        W["wk"] = wb
        bqT = const.tile([128, 2], F32)
        nc.sync.dma_start(out=bqT, in_=bq.rearrange("(m p) -> p m", p=128))
        bkT = const.tile([128, 2], F32)
        nc.sync.dma_start(out=bkT, in_=bk.rearrange("(m p) -> p m", p=128))
        bv_bf = const.tile([1, CW], BF16)
        nc.sync.dma_start(out=bv_bf, in_=bv[None, :])
        wb = persist.tile([128, 4, CW], BF16, tag="Wwv", name="wv")
        nc.sync.dma_start(out=wb, in_=wv.rearrange("c p n -> p c n"))
        W["wv"] = wb
        for sl in range(1, 4):
            nc.sync.dma_start(
                out=srcT_bf[:, :, sl * 512 : (sl + 1) * 512],
                in_=src_v[:, :, sl * 512 : (sl + 1) * 512],
            )

        # --- persistent QKV outputs ---
        QT = [persist.tile([128, S], BF16, tag=f"QT{m}", name=f"QT{m}") for m in range(2)]
        KT = [persist.tile([128, S], BF16, tag=f"KT{m}", name=f"KT{m}") for m in range(2)]
        Vt = [persist.tile([128, HPC * 65], BF16, tag=f"V{sc}", name=f"Vt{sc}") for sc in range(16)]
        bvb = persist.tile([128, CW], BF16, tag="bvb", name="bvb")

        jit_ctr = [0]

        def jit_tag():
            t = f"jit{jit_ctr[0] % 2}"
            jit_ctr[0] += 1
            return t

        def emit_qk(wname, bT, dst, m, st):
            ps = psumA.tile([128, 512], F32, tag=jit_tag(), name=f"qk{wname}{m}{st}")
            for c in range(4):
                nc.tensor.matmul(
                    ps,
                    lhsT=W[wname][:, c, m * 128 : (m + 1) * 128],
                    rhs=srcT_bf[:, c, st * 512 : (st + 1) * 512],
                    start=(c == 0),
                    stop=(c == 3),
                )
            nc.vector.tensor_scalar_add(
                out=dst[:, st * 512 : (st + 1) * 512], in0=ps, scalar1=bT[:, m : m + 1]
            )

        def emit_v(sc):
            ps2 = psumA.tile([128, CW], F32, tag=jit_tag(), name=f"v{sc}")
            for c in range(4):
                nc.tensor.matmul(
                    ps2,
                    lhsT=srcT_bf[:, c, sc * 128 : (sc + 1) * 128],
                    rhs=W["wv"][:, c, :],
                    start=(c == 0),
                    stop=(c == 3),
                )
            nc.vector.memset(Vt[sc].rearrange("p (h e) -> p h e", e=65)[:, :, 64], 1.0)
            nc.vector.tensor_tensor(
                out=Vt[sc].rearrange("p (h e) -> p h e", e=65)[:, :, 0:64],
                in0=ps2.rearrange("p (h e) -> p h e", e=64),
                in1=bvb.rearrange("p (h e) -> p h e", e=64),
                op=mybir.AluOpType.add,
            )

        def Q(m, st):
            return lambda: emit_qk("wq", bqT, QT[m], m, st)

        def K(m, st):
            return lambda: emit_qk("wk", bkT, KT[m], m, st)

        def V(sc):
            return lambda: emit_v(sc)

        # PE warm-up: a dozen throwaway matmuls gated only on the wq DMA keep
        # the tensor engine busy while the rest of the input streams in, so
        # the p-state is fully ramped when the real projections start.
        warm = psumA.tile([128, 128], F32, tag=jit_tag(), name="warm")
        for _ in range(6):
            nc.tensor.matmul(
                warm, lhsT=W["wq"][:, 0, 0:128], rhs=W["wq"][:, 0, 0:128],
                start=True, stop=True,
            )

        # upfront: exactly what unit 0 / kc 0 needs
        emit_qk("wq", bqT, QT[0], 0, 0)
        emit_qk("wk", bkT, KT[0], 0, 0)
        # V bias, broadcast once over 128 partitions (V projection adds it on
        # the copy out of PSUM, saving a rank-1 matmul per V chunk)
        psb = psumA.tile([128, CW], F32, tag=jit_tag(), name="psb")
        nc.tensor.matmul(psb, lhsT=ones_row, rhs=bv_bf, start=True, stop=True)
        nc.vector.tensor_copy(out=bvb, in_=psb)

        # finalize tail for one (unit, hi), dripped into the next unit's PE
        # slack: PE-transpose the SBUF copy of the accumulator into [128, 65]
        # tiles (denominator becomes a per-partition scalar), reciprocal +
        # tensor_scalar multiply, DMA the [q, d] tile into out [2048, 256].
        def fin_tail(cp, pair, qt, hi, u):
            h = pair * 2 + hi
            pt = psumA.tile([128, 4 * 65], F32, tag=jit_tag(), name=f"pt{u}_{hi}")
            for c in range(4):
                nc.tensor.transpose(
                    pt[:, c * 65 : (c + 1) * 65],
                    cp[:, c * 128 : (c + 1) * 128],
                    ident[0:65, 0:65],
                )
            rc = fin.tile([128, 4], F32, tag="rc", name="rc")
            nc.vector.reciprocal(
                out=rc, in_=pt.rearrange("p (c e) -> p c e", e=65)[:, :, 64]
            )
            ot = fin.tile([128, 4, 64], F32, tag="ot", name="ot")
            for c in range(4):
                nc.vector.tensor_scalar_mul(
                    out=ot[:, c, :],
                    in0=pt[:, c * 65 : c * 65 + 64],
                    scalar1=rc[:, c : c + 1],
                )
            nc.sync.dma_start(
                out=out_d[
                    qt * 512 : (qt + 1) * 512, h * 64 : (h + 1) * 64
                ].rearrange("(c p) e -> p c e", p=128),
                in_=ot,
            )

        # drip schedule keyed by global step g (scores/exp of step g and attnV
        # of step g-1 are emitted together; a group at step g lands between
        # exp(g) and attnV(g-1) in PE program order).  Deadlines: V(k) <= step
        # k+1; K(0,st) <= step 4*st - 1; Q/K of later units: unit u first
        # reads at step 16*u.
        sched = {
            1: [V(0)], 2: [V(1)], 3: [V(2), K(0, 1)], 4: [V(3)], 5: [V(4)],
            6: [V(5)], 7: [V(6), K(0, 2)], 8: [V(7)], 9: [V(8)], 10: [V(9)],
            11: [V(10), K(0, 3)], 12: [V(11)], 13: [V(12)], 14: [V(13)],
            15: [V(14), Q(0, 1)], 16: [V(15)],
            19: [Q(0, 2)], 21: [K(1, 0)], 23: [K(1, 1)], 25: [K(1, 2)],
            27: [K(1, 3)], 29: [Q(0, 3)], 35: [Q(1, 0)], 43: [Q(1, 1)],
            51: [Q(1, 2)], 59: [Q(1, 3)],
        }

        # --- attention: flat pipeline over 8 units x 16 kc steps ---
        units = [(pair, qt) for pair in range(2) for qt in range(NQT)]
        accs = {}
        exs = {}
        pend_fin = []
        NSTEP = len(units) * NKC
        LAG = 5
        for g in range(NSTEP + LAG):
            if g < NSTEP:
                u, kc = divmod(g, NKC)
                pair, qt = units[u]
                if kc == 0:
                    accs[u] = [
                        psumA.tile([65, 512], F32, tag=f"acc{hi}", name=f"acc{u}_{hi}")
                        for hi in range(2)
                    ]
                ps = psumS.tile([128, 1024], F32, tag=f"sc{g % 2}", name=f"s{g}")
                for hi in range(2):
                    nc.tensor.matmul(
                        ps[:, hi * 512 : (hi + 1) * 512],
                        lhsT=KT[pair][hi * 64 : (hi + 1) * 64, kc * 128 : (kc + 1) * 128],
                        rhs=QT[pair][hi * 64 : (hi + 1) * 64, qt * 512 : (qt + 1) * 512],
                        start=True,
                        stop=True,
                    )
                if g in OFF_STEPS:
                    ti = expp.tile([128, 1024], I16, tag="expI", name=f"ei{g}")
                    nc.vector.tensor_scalar(
                        out=ti, in0=ps, scalar1=EXP_A, scalar2=EXP_B,
                        op0=mybir.AluOpType.mult, op1=mybir.AluOpType.add,
                    )
                    exs[g] = ti[:].bitcast(BF16)
                else:
                    ex = expp.tile([128, 1024], BF16, tag="expS", name=f"e{g}")
                    nc.scalar.activation(
                        out=ex, in_=ps, func=mybir.ActivationFunctionType.Exp,
                        scale=SCALE,
                    )
                    exs[g] = ex[:]
            for fn in sched.get(g, []):
                fn()
            if pend_fin and g >= 1 and g % NKC in (1, 2):
                pend_fin.pop(0)()
            if g >= LAG:
                up, kcp = divmod(g - LAG, NKC)
                pairp, qtp = units[up]
                exp_ = exs.pop(g - LAG)
                for hi in range(2):
                    h = pairp * 2 + hi
                    nc.tensor.matmul(
                        accs[up][hi],
                        lhsT=Vt[kcp][:, h * 65 : h * 65 + 65],
                        rhs=exp_[:, hi * 512 : (hi + 1) * 512],
                        start=(kcp == 0),
                        stop=(kcp == NKC - 1),
                    )
                if kcp == NKC - 1:
                    # unit done: move accumulators off PSUM, take reciprocal
                    # of the denominator row; the rest of finalize is dripped
                    for hi in range(2):
                        cp = fin.tile([65, 512], F32, tag=f"cp{hi}", name=f"cp{up}_{hi}")
                        nc.vector.tensor_copy(out=cp, in_=accs[up][hi])
                        fn = (lambda cp=cp, pair=pairp, qt=qtp, hi=hi, u=up:
                              fin_tail(cp, pair, qt, hi, u))
                        if up == len(units) - 1:
                            fn()
                        else:
                            pend_fin.append(fn)
                    del accs[up]
        for fn in pend_fin:
            fn()


def build_bass(compile=True):
    # Bacc (not plain Bass): its compile() runs generate_event_semaphores,
    # which splits multi-wait instructions down to the 1-wait-per-instruction
    # hardware limit that walrus enforces.
    nc = bacc.Bacc()
    srcT = nc.declare_dram_parameter("srcT", [D, S], BF16, isOutput=False)
    wq = nc.declare_dram_parameter("wq", [4, 128, CW], BF16, isOutput=False)
    wk = nc.declare_dram_parameter("wk", [4, 128, CW], BF16, isOutput=False)
    wv = nc.declare_dram_parameter("wv", [4, 128, CW], BF16, isOutput=False)
    bq = nc.declare_dram_parameter("bq", [CW], F32, isOutput=False)
    bk = nc.declare_dram_parameter("bk", [CW], F32, isOutput=False)
    bv = nc.declare_dram_parameter("bv", [CW], BF16, isOutput=False)
    out_d = nc.declare_dram_parameter("out", [S, CW], F32, isOutput=True)
    with tile.TileContext(nc) as tc:
        _body(tc, srcT[:], wq[:], wk[:], wv[:], bq[:], bk[:], bv[:], out_d[:])
    if compile:
        nc.compile()
    return nc


_NC = None


def _get_nc():
    global _NC
    if _NC is None:
        _NC = build_bass()
    return _NC


def shard_inputs(inputs):
    src = np.ascontiguousarray(np.asarray(inputs["src"], dtype=np.float32))
    ws = {k: np.asarray(inputs[k], dtype=np.float32) for k in ("Wq", "Wk", "Wv")}
    bs = {k: np.asarray(inputs[k], dtype=np.float32) for k in ("bq", "bk", "bv")}
    in_maps = []
    for c in range(N_CORES):
        b, g = divmod(c, 2)
        cols = slice(g * CW, (g + 1) * CW)
        in_maps.append(
            {
                "srcT": np.ascontiguousarray(src[b].T).astype(BF16_NP),
                "wq": np.ascontiguousarray(ws["Wq"][:, cols]).reshape(4, 128, CW).astype(BF16_NP),
                "wk": np.ascontiguousarray(ws["Wk"][:, cols]).reshape(4, 128, CW).astype(BF16_NP),
                "wv": np.ascontiguousarray(ws["Wv"][:, cols]).reshape(4, 128, CW).astype(BF16_NP),
                "bq": np.ascontiguousarray(bs["bq"][cols]),
                "bk": np.ascontiguousarray(bs["bk"][cols]),
                "bv": np.ascontiguousarray(bs["bv"][cols]).astype(BF16_NP),
            }
        )
    return in_maps


def assemble_output(per_core_outs):
    out = np.empty((B, S, D), np.float32)
    for c in range(N_CORES):
        b, g = divmod(c, 2)
        out[b, :, g * CW : (g + 1) * CW] = per_core_outs[c]
    return out


def run(inputs, trace=False):
    nc = _get_nc()
    in_maps = shard_inputs(inputs)
    res = run_bass_kernel_spmd(nc, in_maps, core_ids=list(range(N_CORES)), trace=trace)
    out = assemble_output([res.results[c]["out"] for c in range(N_CORES)])
    return out, res.exec_time_ns


def kernel(**inputs):
    out, _ = run(inputs)
    return out
